# revision 47
# baseline (speedup 1.0000x reference)
"""Trainium2 Bass kernel: autoregressive graph generator (GNN encoder + LSTM + GNN decoder).

Sharding: 8-way tensor parallel over the LSTM hidden/gate dim. Each core holds
1/8 of the gate rows of W_hh (fp8, SBUF-resident) and computes its slice of the
gates; h is AllGathered (fp8) every step. The encoder SAGEConv (NF=10 -> H=2048)
composed with W_ih factors through a rank-20 bottleneck, so W_ih @ W_enc_{l,r} is
precomposed on the host and the whole x-side becomes a K=20 GEMM per step.
The mean aggregation is a fixed dense matrix A built from edge_index on the host.

Step-level schedule (the collective serializes the recurrence, so the focus is
the ~12us comm window): the gen-step decoder is algebraically folded so
m10 = v'(A^2)^T + w'A^T and x10 = v'A^T + w' come straight from the transposed
decoder output (no x_next round trip), and its x-side contribution is a single
20-row GEMM emitted AFTER the W_hh GEMMs so the whole decoder hides behind the
recurrent matmuls. A scalar-paced dummy-matmul keepalive chain runs through each
collective window to keep the PE HAM clock-gate warm. Constant loads are batched
into 5 DMAs, and the gathered-h HBM->SBUF copies go on two engine queues.

All layouts on device are "T-layout": [feature/hidden dim (partitions), nodes (free)].
"""

import numpy as np
import ml_dtypes

import concourse.mybir as mybir
import concourse.tile as tile
from concourse import bacc, bass_utils
from concourse.bass import ts
from concourse.masks import make_identity

BF = ml_dtypes.bfloat16
F8 = ml_dtypes.float8_e4m3

N, NF, H, NG, K = 256, 10, 2048, 20, 10
NCORES = 8
HS = H // NCORES          # 256 hidden dims per core
GD = 4 * HS               # 1024 gate rows per core
MT = GD // 128            # 8 gate m-tiles per core
KT = H // 128             # 16 h k-tiles
NT = N // 128             # 2 node tiles
GEN = NG - K              # 10 generated steps
WOFF = 8 * 2 * GD         # wdec offset inside pkf8

_PROG = [None]


def _emit_step(nc, pools, consts, t, h_tiles, c_prev, dpool):
    """One LSTM step: gate GEMMs + cell update + AllGather trigger + keepalive.

    hbig k-tile pair j = core j's h2 slice (DoubleRow pairs (2j, 2j+1)).
    Gen steps: decoder front (v' GEMM + transposes) first, then the 64 W_hh
    DR matmuls with the decoder tail (m10/x10) interleaved mid-burst, then a
    single 20-row x-side GEMM, so decoder latency hides behind the W_hh GEMMs.
    Returns (c_new, hbig_for_next_step).
    """
    f32, bf16 = mybir.dt.float32, mybir.dt.bfloat16
    fp8 = mybir.dt.float8e4
    cpool, wpool, apool, gpool, spool = pools
    pk20, pk128, pkf32, bias = (consts["pk20"], consts["pk128"],
                                consts["pkf32"], consts["bias"])
    whhT, wdecT = consts["whhT"], consts["wdecT"]
    ident, xg, out_d = consts["ident"], consts["xg"], consts["out_d"]
    Sig = mybir.ActivationFunctionType.Sigmoid
    Tanh = mybir.ActivationFunctionType.Tanh
    have_h = t > 0
    m_order = [0, 2, 4, 6, 1, 3, 5, 7]  # finish hidden-half 0 (i,f,g,o) early
    DR = mybir.MatmulPerfMode.DoubleRow

    def wc_m(m):
        return pk20[0:20, m * 128:(m + 1) * 128]

    def whh_j(j, m):
        return whhT[j][:].rearrange("p (s w) -> p s w", s=2)[:, :, ts(m, 128)]

    def wdec_j(j):
        return wdecT[j][:].rearrange("p (s w) -> p s w", s=2)

    def at_k(k):
        return pk128[:, ts(k, N)]

    def a2t_k(k):
        return pk128[:, ts(2 + k, N)]

    hview = h_tiles[:].rearrange("p (a n) -> p a n", a=KT) if have_h else None
    gen = t >= K
    vw_sb, vwT = None, None

    if gen:
        s = t - K
        L1, L2, L3 = consts["L1"], consts["L2"], consts["L3"]
        # decoder front: v'|w' = Wdec^T h  [16, N], then +qr, then transpose
        vw_ps = spool.tile([16, N], f32, tag="sp", name=f"vwps{t}")
        for j in range(KT // 2):
            nc.tensor.matmul(vw_ps[:], wdec_j(j),
                             hview[:, 2 * j:2 * j + 2, :],
                             start=(j == 0), stop=(j == KT // 2 - 1),
                             perf_mode=DR)
        vw_sb = wpool.tile([16, N], bf16, tag="vw", name=f"vwsb{t}")
        nc.vector.tensor_scalar_add(vw_sb[:], vw_ps[:],
                                    pkf32[0:16, 4 + s:5 + s])
        # scatter v'/w' transposes into the L staging tiles (column slices
        # only -- partition-aligned).  L1=[v|0|0], L2=[w|v|0], L3=[0|w|cst].
        for j in range(NT):
            tp = spool.tile([128, 16], bf16, tag="sp", name=f"vwT{t}_{j}")
            nc.tensor.transpose(tp[:], vw_sb[:, ts(j, 128)], ident[:16, :16])
            nc.vector.tensor_copy(L1[j][:, 0:8], tp[:, 0:8])
            nc.vector.tensor_copy(L2[j][:, 0:8], tp[:, 8:16])
            nc.vector.tensor_copy(L2[j][:, 8:16], tp[:, 0:8])
            nc.vector.tensor_copy(L3[j][:, 8:16], tp[:, 8:16])

    # recurrent gate GEMMs.  PSUM banks pair m-tiles that finish
    # consecutively in m_order ((0,2),(4,6),(1,3),(5,7)) so the hh=0
    # activations fire as soon as their bank closes.
    # Warmup: all 8 x-side MMs (host-precomputed r20w, no h dependency) are
    # emitted BEFORE the W_hh loop so they execute inside the collective
    # window while the PE is otherwise idle.  Gen: W_hh first (start),
    # x-side last (stop) once the decoder tail lands.
    gp = {}
    for mi, m in enumerate(m_order):
        g_t = gpool.tile([128, N], f32, tag="gp", name=f"gp{t}_{m}")
        gp[m] = g_t[:]
        g = gp[m]
        if not gen:
            nc.tensor.matmul(g, wc_m(m),
                             pk20[0:20, GD + t * N:GD + (t + 1) * N],
                             start=True, stop=not have_h)
        if have_h:
            for j in range(KT // 2):
                nc.tensor.matmul(
                    g, whh_j(j, m), hview[:, 2 * j:2 * j + 2, :],
                    start=gen and (j == 0),
                    stop=(not gen) and (j == KT // 2 - 1),
                    perf_mode=DR)
        if gen and mi == 3:
            # decoder tail mid-burst, one [20, N] accumulation:
            # rows 0:8  = m10' = v'(A^2)^T + w'A^T      (L1 x a2t + L2 x at)
            # rows 8:16 = x10' = v'A^T + w'             (L2 x at + L3 x ident)
            # rows 16:20 = [(A@st2)^T ; st2^T]          (L3 const cols x ident)
            L1, L2, L3 = consts["L1"], consts["L2"], consts["L3"]
            out20 = spool.tile([20, N], f32, tag="sp", name=f"o20{t}")
            for j in range(NT):
                nc.tensor.matmul(out20[:], L1[j][:], a2t_k(j),
                                 start=(j == 0), stop=False)
                nc.tensor.matmul(out20[:], L2[j][:], at_k(j),
                                 start=False, stop=False)
            for j in range(NT):
                nc.tensor.matmul(out20[:, ts(j, 128)], L3[j][:], ident[:],
                                 start=False, stop=(j == NT - 1))
            nc.vector.tensor_copy(xg[:], out20[:])
            # f32 copy for the output path (avoids bf16-rounding the result)
            xgf = consts["xgf"]
            nc.vector.tensor_copy(xgf[:], out20[:])
        if gen and mi >= 4:
            # xg is ready once the mi==3 decoder tail lands: close gates as
            # the burst progresses (one early hh0 gate + the current one) so
            # the activations interleave instead of bunching at burst end
            for mc in (m_order[mi - 4], m):
                nc.tensor.matmul(gp[mc], wc_m(mc), xg[:],
                                 start=False, stop=True)

    h2 = apool.tile([128, 2 * N], fp8, tag="h2", name=f"h2_{t}")
    hb = None
    if t < NG - 1:
        hb = wpool.tile([128, KT * N], fp8, tag="hbig", name=f"hbig{t}")
    c_new = []
    for hh in range(2):
        def bcol(m):
            return bias[:, (m * NG + t):(m * NG + t + 1)]
        isc = 1.0 / 64.0     # undo the x64 fp8-normal-range weight scaling
        si = apool.tile([128, N], f32, tag="si", name=f"si{t}_{hh}")
        nc.scalar.activation(si[:], gp[0 + hh], Sig, bias=bcol(0 + hh), scale=isc)
        sf = apool.tile([128, N], f32, tag="sf", name=f"sf{t}_{hh}")
        nc.scalar.activation(sf[:], gp[2 + hh], Sig, bias=bcol(2 + hh), scale=isc)
        tg = apool.tile([128, N], f32, tag="tg", name=f"tg{t}_{hh}")
        nc.scalar.activation(tg[:], gp[4 + hh], Tanh, bias=bcol(4 + hh), scale=isc)
        so = apool.tile([128, N], f32, tag="so", name=f"so{t}_{hh}")
        nc.scalar.activation(so[:], gp[6 + hh], Sig, bias=bcol(6 + hh), scale=isc)

        cn = wpool.tile([128, N], f32, tag=f"c{hh}", name=f"c{t}_{hh}")
        if t == 0:
            nc.vector.tensor_mul(cn[:], si[:], tg[:])          # c = sig(i)*tanh(g)
        else:
            p = apool.tile([128, N], f32, tag="p", name=f"p{t}_{hh}")
            nc.vector.tensor_mul(p[:], si[:], tg[:])
            tmp = apool.tile([128, N], f32, tag="tmp", name=f"tmp{t}_{hh}")
            nc.vector.tensor_mul(tmp[:], sf[:], c_prev[hh][:])
            nc.vector.tensor_add(cn[:], tmp[:], p[:])
        tc2 = apool.tile([128, N], f32, tag="tc", name=f"tc{t}_{hh}")
        nc.scalar.activation(tc2[:], cn[:], Tanh)
        nc.vector.tensor_mul(h2[:, ts(hh, N)], so[:], tc2[:])
        c_new.append(cn)

    if t < NG - 1:
        inb = dpool.tile([N, N], fp8, tag="inb", name=f"inb{t}")
        outb = dpool.tile([H, N], fp8, tag="outb", name=f"outb{t}")
        # per-half input DMAs: the hh=0 half ships while the hh=1
        # elementwise is still running
        for hh in range(2):
            nc.sync.dma_start(inb[ts(hh, 128), :], h2[:, ts(hh, N)])
        nc.gpsimd.collective_compute(
            "AllGather",
            mybir.AluOpType.bypass,
            replica_groups=[list(range(NCORES))],
            ins=[inb.opt()],
            outs=[outb.opt()],
        )
        outb3 = outb.rearrange("(a p) n -> p a n", p=128)
        # gathered-h copies in 4 chunks on two engine queues: the first
        # chunk (pairs j=0,1) lands earlier so the next step's W_hh matmuls
        # start sooner; later chunks stream in behind the consuming j-loop
        engines = [nc.sync, nc.scalar, nc.sync, nc.scalar]
        for q in range(4):
            engines[q].dma_start(
                hb[:, ts(q, KT * N // 4)].rearrange(
                    "p (a n) -> p a n", a=KT // 4),
                outb3[:, ts(q, KT // 4), :])

    if gen:
        # output x_next = [st2 | x10'^T], off the critical path
        xgf, identf = consts["xgf"], consts["identf"]
        for j in range(NT):
            xt = spool.tile([128, 20], f32, tag="sp", name=f"xt{t}_{j}")
            nc.tensor.transpose(xt[:], xgf[0:20, ts(j, 128)], identf[:20, :20])
            xn = wpool.tile([128, NF], f32, tag=f"xn{j}", name=f"xn{t}_{j}")
            nc.vector.tensor_copy(xn[:, 0:2], pkf32[:, 2 * j:2 * j + 2])
            nc.scalar.activation(xn[:, 2:NF], xt[:, 8:16],
                                 mybir.ActivationFunctionType.Copy,
                                 scale=1.0 / 64.0)
            nc.sync.dma_start(out_d[t - K, ts(j, 128), :], xn[:])
    return c_new, hb


def _build_program():
    f32, bf16 = mybir.dt.float32, mybir.dt.bfloat16
    fp8 = mybir.dt.float8e4
    nc = bacc.Bacc("TRN2", target_bir_lowering=False, debug=False,
                   num_devices=NCORES)

    # batched constant inputs (few big DMAs instead of ~26 serialized triggers)
    pk20_d = nc.dram_tensor("pk20", [20, GD + K * N], bf16,
                            kind="ExternalInput").ap()          # wc | r20w
    bias_d = nc.dram_tensor("biases", [128, MT * NG], f32,
                            kind="ExternalInput").ap()
    pkf8_d = nc.dram_tensor("pkf8", [128, WOFF + 8 * 32], fp8,
                            kind="ExternalInput").ap()          # whh | wdec
    pk128_d = nc.dram_tensor("pk128", [128, 4 * N + 8], bf16,
                             kind="ExternalInput").ap()         # at | a2t | cst
    pkf32_d = nc.dram_tensor("pkf32", [128, 4 + GEN], f32,
                             kind="ExternalInput").ap()         # st2 | qr
    out_d = nc.dram_tensor("gen", [GEN, N, NF], f32, kind="ExternalOutput").ap()
    hdbg_d = nc.dram_tensor("hdbg", [128, KT * N], fp8,
                            kind="ExternalOutput").ap()

    with tile.TileContext(nc) as tc:
        with (
            tc.tile_pool(name="const", bufs=1) as cpool,
            tc.tile_pool(name="work", bufs=2) as wpool,
            tc.tile_pool(name="act", bufs=3) as apool,
            tc.tile_pool(name="gp", bufs=6, space="PSUM") as gpool,
            tc.tile_pool(name="sp", bufs=2, space="PSUM") as spool,
            tc.tile_pool(name="dram", bufs=2, space="DRAM") as dpool,
        ):
            pools = (cpool, wpool, apool, gpool, spool)

            # ---- constants, batched.  Order: step-0 needs pk20+bias first.
            pk20 = cpool.tile([20, GD + K * N], bf16, tag="pk20", name="pk20")
            nc.sync.dma_start(pk20[:], pk20_d[:])
            bias = cpool.tile([128, MT * NG], f32, tag="bias", name="bias")
            nc.sync.dma_start(bias[:], bias_d[:])
            whhT = []
            for k in range(KT // 2):
                w = cpool.tile([128, 2 * GD], fp8, tag=f"whh{k}", name=f"whh{k}")
                nc.scalar.dma_start(w[:], pkf8_d[:, k * 2 * GD:(k + 1) * 2 * GD])
                whhT.append(w)
            wdecT = []
            for k in range(KT // 2):
                w = cpool.tile([128, 32], fp8, tag=f"wdec{k}", name=f"wdec{k}")
                nc.scalar.dma_start(
                    w[:], pkf8_d[:, WOFF + 32 * k:WOFF + 32 * (k + 1)])
                wdecT.append(w)
            pk128 = cpool.tile([128, 4 * N + 8], bf16, tag="pk128",
                               name="pk128")
            nc.sync.dma_start(pk128[:], pk128_d[:])
            pkf32 = cpool.tile([128, 4 + GEN], f32, tag="pkf32",
                               name="pkf32")
            nc.sync.dma_start(pkf32[:], pkf32_d[:])

            ident = cpool.tile([128, 128], bf16, tag="ident", name="ident")
            make_identity(nc, ident[:])
            identf = cpool.tile([32, 32], f32, tag="identf", name="identf")
            make_identity(nc, identf[:])

            # decoder staging tiles: L1=[v|0|0], L2=[w|v|0], L3=[0|w|cst]
            # (zero/const columns written once; v/w columns per gen step)
            xg = cpool.tile([20, N], bf16, tag="xg", name="xg")
            xgf = cpool.tile([20, N], f32, tag="xgf", name="xgf")
            L1, L2, L3 = [], [], []
            for j in range(NT):
                for name, lst in (("L1", L1), ("L2", L2), ("L3", L3)):
                    lt = cpool.tile([128, 20], bf16, tag=f"{name}_{j}",
                                    name=f"{name}_{j}")
                    nc.gpsimd.memset(lt[:], 0.0)
                    lst.append(lt)
                nc.vector.tensor_copy(
                    L3[j][:, 16:20], pk128[:, 4 * N + 4 * j:4 * N + 4 * j + 4])

            consts = dict(pk20=pk20, whhT=whhT, wdecT=wdecT, pk128=pk128,
                          pkf32=pkf32,
                          bias=bias, ident=ident, identf=identf, xg=xg,
                          xgf=xgf, out_d=out_d, L1=L1, L2=L2, L3=L3)

            h_tiles, c_prev = None, None
            for t in range(NG):
                if t == K:
                    nc.sync.dma_start(hdbg_d, h_tiles[:])
                c_prev, h_tiles = _emit_step(nc, pools, consts, t, h_tiles,
                                             c_prev, dpool)
    nc.compile()
    return nc


def _host_tensors(inputs):
    """All host-side preprocessing: A matrix, weight composition, per-core shards."""
    f32 = np.float32
    kg = np.asarray(inputs["known_graphs"], f32)
    ei = np.asarray(inputs["edge_index"])
    W_enc_l = np.asarray(inputs["W_enc_l"], f32)
    b_enc_l = np.asarray(inputs["b_enc_l"], f32)
    W_enc_r = np.asarray(inputs["W_enc_r"], f32)
    pos = np.asarray(inputs["pos_emb"], f32)
    W_ih = np.asarray(inputs["W_ih"], f32)
    W_hh = np.asarray(inputs["W_hh"], f32)
    b_ih = np.asarray(inputs["b_ih"], f32)
    b_hh = np.asarray(inputs["b_hh"], f32)
    W_dec_l = np.asarray(inputs["W_dec_l"], f32)
    b_dec_l = np.asarray(inputs["b_dec_l"], f32)
    W_dec_r = np.asarray(inputs["W_dec_r"], f32)

    src, dst = np.asarray(ei[0]), np.asarray(ei[1])
    C = np.zeros((N, N), np.float64)
    np.add.at(C, (dst, src), 1.0)
    cnt = C.sum(1)
    A = (C / np.maximum(cnt, 1.0)[:, None]).astype(f32)

    c64 = np.float64
    Wc1 = W_ih.astype(c64) @ W_enc_l.astype(c64)          # [4H, NF]
    Wc2 = W_ih.astype(c64) @ W_enc_r.astype(c64)
    Wc = np.concatenate([Wc1, Wc2], 1)                    # [4H, 20]
    # bias_t = W_ih @ (b_enc_l + pe_t) + b_ih + b_hh  -> [NG, 4H]
    bias_all = (W_ih.astype(c64) @ (b_enc_l.astype(c64)[:, None] + pos.astype(c64).T)).T \
        + b_ih.astype(c64) + b_hh.astype(c64)
    bias_all = bias_all.astype(f32)
    # decoder pe folds: [16, GEN].  x4096: the whole pre-activation domain is
    # scaled by 64 (h) x 64 (fp8 weights) to stay in fp8e4m3's normal range.
    SC = 64.0
    qr = (np.concatenate([
        (pos[K:NG].astype(c64) @ W_dec_l.T.astype(c64)).T,
        (pos[K:NG].astype(c64) @ W_dec_r.T.astype(c64)).T
        + b_dec_l.astype(c64)[:, None],
    ], 0) * SC).astype(f32)

    # warm-up rhs20: [20, K*N], col index t*N + i
    mean_w = np.einsum("ij,tjf->tif", A.astype(c64), kg.astype(c64))  # [K, N, NF]
    r20w = np.concatenate([
        np.transpose(mean_w, (2, 0, 1)).reshape(NF, -1),
        np.transpose(kg.astype(c64), (2, 0, 1)).reshape(NF, -1),
    ], 0).astype(f32)

    # DoubleRow pair packing: [KT/2 * 128, 2*cols], row j*128+p holds
    # global k-tiles (2j, 2j+1) side by side along the free dim
    def pack_pairs(wT):  # wT [H, cols] -> [H/2, 2*cols]
        cols = wT.shape[1]
        return np.ascontiguousarray(
            wT.reshape(KT // 2, 2, 128, cols).transpose(0, 2, 1, 3)
            .reshape(H // 2, 2 * cols))

    def to_sbuf_cols(x, rows=128):
        # [n*rows, cols] -> [rows, n*cols]: row-block k becomes column-block k
        n = x.shape[0] // rows
        return np.ascontiguousarray(
            x.reshape(n, rows, x.shape[1]).transpose(1, 0, 2)
            .reshape(rows, n * x.shape[1]))

    st2 = kg[-1, :, :2]                                    # [N, 2]
    a_st2 = (A.astype(c64) @ st2.astype(c64)).astype(f32)  # [N, 2]
    A2 = (A.astype(c64) @ A.astype(c64)).astype(f32)

    # xg row order: [mean f2:10 | root f2:10 | mean f0:2 | root f0:2];
    # the same permutation applies to the wc rows and (for the warmup GEMM,
    # where the sum over rows is order-invariant) to r20w.
    perm = list(range(2, 10)) + list(range(12, 20)) + [0, 1, 10, 11]

    # pk128: at(2N) | a2t(2N) | cst(8)  [128, 4N+8] bf16
    cst = np.zeros((128, 8), f32)
    for j in range(NT):
        cst[:, 4 * j:4 * j + 2] = SC * a_st2[128 * j:128 * (j + 1)]
        cst[:, 4 * j + 2:4 * j + 4] = SC * st2[128 * j:128 * (j + 1)]
    pk128 = np.concatenate(
        [to_sbuf_cols(np.ascontiguousarray(A.T)),
         to_sbuf_cols(np.ascontiguousarray(A2.T)), cst], 1).astype(BF)

    # pkf32: st2 cols(4) | qr(GEN cols, rows 0:16)
    pkf32 = np.zeros((128, 4 + GEN), f32)
    pkf32[:, 0:2] = st2[:128]
    pkf32[:, 2:4] = st2[128:]
    pkf32[0:16, 4:4 + GEN] = qr

    wdecT = np.concatenate([W_dec_l, W_dec_r], 0).T * 64.0  # [H, 16]
    wdec_packed = to_sbuf_cols(pack_pairs(wdecT)).astype(F8)  # [128, 8*32]

    shared = {
        "pk128": pk128,
        "pkf32": pkf32,
    }

    in_maps = []
    for c in range(NCORES):
        idx = np.concatenate([np.arange(g * H + c * HS, g * H + (c + 1) * HS)
                              for g in range(4)])
        whhT = pack_pairs(W_hh[idx, :].T * 64.0).astype(F8)           # [H/2, 2GD]
        pkf8 = np.concatenate([to_sbuf_cols(whhT), wdec_packed], 1)
        wcT = np.ascontiguousarray(Wc[idx, :].T[perm]).astype(BF)     # [20, GD]
        pk20 = np.concatenate([wcT, (SC * r20w[perm]).astype(BF)], 1)
        bc = bias_all[:, idx].T                                       # [GD, NG]
        bt = np.ascontiguousarray(
            bc.reshape(MT, 128, NG).transpose(1, 0, 2).reshape(128, MT * NG))
        in_maps.append({
            "pkf8": pkf8, "pk20": pk20, "biases": bt, **shared,
        })
    return in_maps


def kernel(**inputs):
    if _PROG[0] is None:
        _PROG[0] = _build_program()
    nc = _PROG[0]
    in_maps = _host_tensors(inputs)
    res = bass_utils.run_bass_kernel_spmd(
        nc, in_maps, core_ids=list(range(NCORES)))
    kernel.hdbg = np.ascontiguousarray(res.results[0]["hdbg"])
    return np.ascontiguousarray(res.results[0]["gen"]).astype(np.float32)


# exposed for test.py profiling
def run_profiled(inputs, **kwargs):
    if _PROG[0] is None:
        _PROG[0] = _build_program()
    in_maps = _host_tensors(inputs)
    return bass_utils.run_bass_kernel_spmd(
        _PROG[0], in_maps, core_ids=list(range(NCORES)), **kwargs)


# revision 48
# speedup vs baseline: 1.0071x; 1.0071x over previous
"""Trainium2 Bass kernel: autoregressive graph generator (GNN encoder + LSTM + GNN decoder).

Sharding: 8-way tensor parallel over the LSTM hidden/gate dim. Each core holds
1/8 of the gate rows of W_hh (fp8, SBUF-resident) and computes its slice of the
gates; h is AllGathered (fp8) every step. The encoder SAGEConv (NF=10 -> H=2048)
composed with W_ih factors through a rank-20 bottleneck, so W_ih @ W_enc_{l,r} is
precomposed on the host and the whole x-side becomes a K=20 GEMM per step.
The mean aggregation is a fixed dense matrix A built from edge_index on the host.

Step-level schedule (the collective serializes the recurrence, so the focus is
the ~12us comm window): the gen-step decoder is algebraically folded so
m10 = v'(A^2)^T + w'A^T and x10 = v'A^T + w' come straight from the transposed
decoder output (no x_next round trip), and its x-side contribution is a single
20-row GEMM emitted AFTER the W_hh GEMMs so the whole decoder hides behind the
recurrent matmuls. A scalar-paced dummy-matmul keepalive chain runs through each
collective window to keep the PE HAM clock-gate warm. Constant loads are batched
into 5 DMAs, and the gathered-h HBM->SBUF copies go on two engine queues.

All layouts on device are "T-layout": [feature/hidden dim (partitions), nodes (free)].
"""

import numpy as np
import ml_dtypes

import concourse.mybir as mybir
import concourse.tile as tile
from concourse import bacc, bass_utils
from concourse.bass import ts
from concourse.masks import make_identity

BF = ml_dtypes.bfloat16
F8 = ml_dtypes.float8_e4m3

N, NF, H, NG, K = 256, 10, 2048, 20, 10
NCORES = 8
HS = H // NCORES          # 256 hidden dims per core
GD = 4 * HS               # 1024 gate rows per core
MT = GD // 128            # 8 gate m-tiles per core
KT = H // 128             # 16 h k-tiles
NT = N // 128             # 2 node tiles
GEN = NG - K              # 10 generated steps
WOFF = 8 * 2 * GD         # wdec offset inside pkf8

_PROG = [None]


def _emit_step(nc, pools, consts, t, h_tiles, c_prev, dpool):
    """One LSTM step: gate GEMMs + cell update + AllGather trigger + keepalive.

    hbig k-tile pair j = core j's h2 slice (DoubleRow pairs (2j, 2j+1)).
    Gen steps: decoder front (v' GEMM + transposes) first, then the 64 W_hh
    DR matmuls with the decoder tail (m10/x10) interleaved mid-burst, then a
    single 20-row x-side GEMM, so decoder latency hides behind the W_hh GEMMs.
    Returns (c_new, hbig_for_next_step).
    """
    f32, bf16 = mybir.dt.float32, mybir.dt.bfloat16
    fp8 = mybir.dt.float8e4
    cpool, wpool, apool, gpool, spool = pools
    pk20, pk128, pkf32, bias = (consts["pk20"], consts["pk128"],
                                consts["pkf32"], consts["bias"])
    whhT, wdecT = consts["whhT"], consts["wdecT"]
    ident, xg, out_d = consts["ident"], consts["xg"], consts["out_d"]
    Sig = mybir.ActivationFunctionType.Sigmoid
    Tanh = mybir.ActivationFunctionType.Tanh
    have_h = t > 0
    m_order = [0, 2, 4, 6, 1, 3, 5, 7]  # finish hidden-half 0 (i,f,g,o) early
    DR = mybir.MatmulPerfMode.DoubleRow

    def wc_m(m):
        return pk20[0:20, m * 128:(m + 1) * 128]

    def whh_j(j, m):
        return whhT[j][:].rearrange("p (s w) -> p s w", s=2)[:, :, ts(m, 128)]

    def wdec_j(j):
        return wdecT[j][:].rearrange("p (s w) -> p s w", s=2)

    def at_k(k):
        return pk128[:, ts(k, N)]

    def a2t_k(k):
        return pk128[:, ts(2 + k, N)]

    hview = h_tiles[:].rearrange("p (a n) -> p a n", a=KT) if have_h else None
    gen = t >= K
    vw_sb, vwT = None, None

    if gen:
        s = t - K
        L1, L2, L3 = consts["L1"], consts["L2"], consts["L3"]
        # decoder front: v'|w' = Wdec^T h  [16, N], then +qr, then transpose
        vw_ps = spool.tile([16, N], f32, tag="sp", name=f"vwps{t}")
        for j in range(KT // 2):
            nc.tensor.matmul(vw_ps[:], wdec_j(j),
                             hview[:, 2 * j:2 * j + 2, :],
                             start=(j == 0), stop=(j == KT // 2 - 1),
                             perf_mode=DR)
        vw_sb = wpool.tile([16, N], bf16, tag="vw", name=f"vwsb{t}")
        nc.vector.tensor_scalar_add(vw_sb[:], vw_ps[:],
                                    pkf32[0:16, 4 + s:5 + s])
        # scatter v'/w' transposes into the L staging tiles (column slices
        # only -- partition-aligned).  L1=[v|0|0], L2=[w|v|0], L3=[0|w|cst].
        for j in range(NT):
            tp = spool.tile([128, 16], bf16, tag="sp", name=f"vwT{t}_{j}")
            nc.tensor.transpose(tp[:], vw_sb[:, ts(j, 128)], ident[:16, :16])
            nc.vector.tensor_copy(L1[j][:, 0:8], tp[:, 0:8])
            nc.vector.tensor_copy(L2[j][:, 0:8], tp[:, 8:16])
            nc.vector.tensor_copy(L2[j][:, 8:16], tp[:, 0:8])
            nc.vector.tensor_copy(L3[j][:, 8:16], tp[:, 8:16])

    # recurrent gate GEMMs.  PSUM banks pair m-tiles that finish
    # consecutively in m_order ((0,2),(4,6),(1,3),(5,7)) so the hh=0
    # activations fire as soon as their bank closes.
    # Warmup: all 8 x-side MMs (host-precomputed r20w, no h dependency) are
    # emitted BEFORE the W_hh loop so they execute inside the collective
    # window while the PE is otherwise idle.  Gen: W_hh first (start),
    # x-side last (stop) once the decoder tail lands.
    gp = {}
    for mi, m in enumerate(m_order):
        g_t = gpool.tile([128, N], f32, tag="gp", name=f"gp{t}_{m}")
        gp[m] = g_t[:]
        g = gp[m]
        if not gen:
            nc.tensor.matmul(g, wc_m(m),
                             pk20[0:20, GD + t * N:GD + (t + 1) * N],
                             start=True, stop=not have_h)
        if have_h:
            for j in range(KT // 2):
                nc.tensor.matmul(
                    g, whh_j(j, m), hview[:, 2 * j:2 * j + 2, :],
                    start=gen and (j == 0),
                    stop=(not gen) and (j == KT // 2 - 1),
                    perf_mode=DR)
        if gen and mi == 3:
            # decoder tail mid-burst, one [20, N] accumulation:
            # rows 0:8  = m10' = v'(A^2)^T + w'A^T      (L1 x a2t + L2 x at)
            # rows 8:16 = x10' = v'A^T + w'             (L2 x at + L3 x ident)
            # rows 16:20 = [(A@st2)^T ; st2^T]          (L3 const cols x ident)
            L1, L2, L3 = consts["L1"], consts["L2"], consts["L3"]
            out20 = spool.tile([20, N], f32, tag="sp", name=f"o20{t}")
            for j in range(NT):
                nc.tensor.matmul(out20[:], L1[j][:], a2t_k(j),
                                 start=(j == 0), stop=False)
                nc.tensor.matmul(out20[:], L2[j][:], at_k(j),
                                 start=False, stop=False)
            for j in range(NT):
                nc.tensor.matmul(out20[:, ts(j, 128)], L3[j][:], ident[:],
                                 start=False, stop=(j == NT - 1))
            nc.vector.tensor_copy(xg[:], out20[:])
            # f32 copy for the output path (avoids bf16-rounding the result)
            xgf = consts["xgf"]
            nc.vector.tensor_copy(xgf[:], out20[:])

    if gen:
        for m in m_order:
            nc.tensor.matmul(gp[m], wc_m(m), xg[:],
                             start=False, stop=True)

    h2 = apool.tile([128, 2 * N], fp8, tag="h2", name=f"h2_{t}")
    hb = None
    if t < NG - 1:
        hb = wpool.tile([128, KT * N], fp8, tag="hbig", name=f"hbig{t}")
    c_new = []
    for hh in range(2):
        def bcol(m):
            return bias[:, (m * NG + t):(m * NG + t + 1)]
        isc = 1.0 / 64.0     # undo the x64 fp8-normal-range weight scaling
        si = apool.tile([128, N], f32, tag="si", name=f"si{t}_{hh}")
        nc.scalar.activation(si[:], gp[0 + hh], Sig, bias=bcol(0 + hh), scale=isc)
        sf = apool.tile([128, N], f32, tag="sf", name=f"sf{t}_{hh}")
        nc.scalar.activation(sf[:], gp[2 + hh], Sig, bias=bcol(2 + hh), scale=isc)
        tg = apool.tile([128, N], f32, tag="tg", name=f"tg{t}_{hh}")
        nc.scalar.activation(tg[:], gp[4 + hh], Tanh, bias=bcol(4 + hh), scale=isc)
        so = apool.tile([128, N], f32, tag="so", name=f"so{t}_{hh}")
        nc.scalar.activation(so[:], gp[6 + hh], Sig, bias=bcol(6 + hh), scale=isc)

        cn = wpool.tile([128, N], f32, tag=f"c{hh}", name=f"c{t}_{hh}")
        if t == 0:
            nc.vector.tensor_mul(cn[:], si[:], tg[:])          # c = sig(i)*tanh(g)
        else:
            p = apool.tile([128, N], f32, tag="p", name=f"p{t}_{hh}")
            nc.vector.tensor_mul(p[:], si[:], tg[:])
            tmp = apool.tile([128, N], f32, tag="tmp", name=f"tmp{t}_{hh}")
            nc.vector.tensor_mul(tmp[:], sf[:], c_prev[hh][:])
            nc.vector.tensor_add(cn[:], tmp[:], p[:])
        tc2 = apool.tile([128, N], f32, tag="tc", name=f"tc{t}_{hh}")
        nc.scalar.activation(tc2[:], cn[:], Tanh)
        nc.vector.tensor_mul(h2[:, ts(hh, N)], so[:], tc2[:])
        c_new.append(cn)

    if t < NG - 1:
        inb = dpool.tile([N, N], fp8, tag="inb", name=f"inb{t}")
        outb = dpool.tile([H, N], fp8, tag="outb", name=f"outb{t}")
        # per-half input DMAs: the hh=0 half ships while the hh=1
        # elementwise is still running
        for hh in range(2):
            nc.sync.dma_start(inb[ts(hh, 128), :], h2[:, ts(hh, N)])
        nc.gpsimd.collective_compute(
            "AllGather",
            mybir.AluOpType.bypass,
            replica_groups=[list(range(NCORES))],
            ins=[inb.opt()],
            outs=[outb.opt()],
        )
        outb3 = outb.rearrange("(a p) n -> p a n", p=128)
        # gathered-h copies in 4 chunks on two engine queues: the first
        # chunk (pairs j=0,1) lands earlier so the next step's W_hh matmuls
        # start sooner; later chunks stream in behind the consuming j-loop
        engines = [nc.sync, nc.scalar, nc.sync, nc.scalar]
        for q in range(4):
            engines[q].dma_start(
                hb[:, ts(q, KT * N // 4)].rearrange(
                    "p (a n) -> p a n", a=KT // 4),
                outb3[:, ts(q, KT // 4), :])

    if gen:
        # output x_next = [st2 | x10'^T], off the critical path
        xgf, identf = consts["xgf"], consts["identf"]
        for j in range(NT):
            xt = spool.tile([128, 20], f32, tag="sp", name=f"xt{t}_{j}")
            nc.tensor.transpose(xt[:], xgf[0:20, ts(j, 128)], identf[:20, :20])
            xn = wpool.tile([128, NF], f32, tag=f"xn{j}", name=f"xn{t}_{j}")
            nc.vector.tensor_copy(xn[:, 0:2], pkf32[:, 2 * j:2 * j + 2])
            nc.scalar.activation(xn[:, 2:NF], xt[:, 8:16],
                                 mybir.ActivationFunctionType.Copy,
                                 scale=1.0 / 64.0)
            nc.sync.dma_start(out_d[t - K, ts(j, 128), :], xn[:])
    return c_new, hb


def _build_program():
    f32, bf16 = mybir.dt.float32, mybir.dt.bfloat16
    fp8 = mybir.dt.float8e4
    nc = bacc.Bacc("TRN2", target_bir_lowering=False, debug=False,
                   num_devices=NCORES)

    # batched constant inputs (few big DMAs instead of ~26 serialized triggers)
    pk20_d = nc.dram_tensor("pk20", [20, GD + K * N], bf16,
                            kind="ExternalInput").ap()          # wc | r20w
    bias_d = nc.dram_tensor("biases", [128, MT * NG], f32,
                            kind="ExternalInput").ap()
    pkf8_d = nc.dram_tensor("pkf8", [128, WOFF + 8 * 32], fp8,
                            kind="ExternalInput").ap()          # whh | wdec
    pk128_d = nc.dram_tensor("pk128", [128, 4 * N + 8], bf16,
                             kind="ExternalInput").ap()         # at | a2t | cst
    pkf32_d = nc.dram_tensor("pkf32", [128, 4 + GEN], f32,
                             kind="ExternalInput").ap()         # st2 | qr
    out_d = nc.dram_tensor("gen", [GEN, N, NF], f32, kind="ExternalOutput").ap()
    hdbg_d = nc.dram_tensor("hdbg", [128, KT * N], fp8,
                            kind="ExternalOutput").ap()

    with tile.TileContext(nc) as tc:
        with (
            tc.tile_pool(name="const", bufs=1) as cpool,
            tc.tile_pool(name="work", bufs=2) as wpool,
            tc.tile_pool(name="act", bufs=3) as apool,
            tc.tile_pool(name="gp", bufs=6, space="PSUM") as gpool,
            tc.tile_pool(name="sp", bufs=2, space="PSUM") as spool,
            tc.tile_pool(name="dram", bufs=2, space="DRAM") as dpool,
        ):
            pools = (cpool, wpool, apool, gpool, spool)

            # ---- constants, batched.  Order: step-0 needs pk20+bias first.
            pk20 = cpool.tile([20, GD + K * N], bf16, tag="pk20", name="pk20")
            nc.sync.dma_start(pk20[:], pk20_d[:])
            bias = cpool.tile([128, MT * NG], f32, tag="bias", name="bias")
            nc.sync.dma_start(bias[:], bias_d[:])
            whhT = []
            for k in range(KT // 2):
                w = cpool.tile([128, 2 * GD], fp8, tag=f"whh{k}", name=f"whh{k}")
                nc.scalar.dma_start(w[:], pkf8_d[:, k * 2 * GD:(k + 1) * 2 * GD])
                whhT.append(w)
            wdecT = []
            for k in range(KT // 2):
                w = cpool.tile([128, 32], fp8, tag=f"wdec{k}", name=f"wdec{k}")
                nc.scalar.dma_start(
                    w[:], pkf8_d[:, WOFF + 32 * k:WOFF + 32 * (k + 1)])
                wdecT.append(w)
            pk128 = cpool.tile([128, 4 * N + 8], bf16, tag="pk128",
                               name="pk128")
            nc.sync.dma_start(pk128[:], pk128_d[:])
            pkf32 = cpool.tile([128, 4 + GEN], f32, tag="pkf32",
                               name="pkf32")
            nc.sync.dma_start(pkf32[:], pkf32_d[:])

            ident = cpool.tile([128, 128], bf16, tag="ident", name="ident")
            make_identity(nc, ident[:])
            identf = cpool.tile([32, 32], f32, tag="identf", name="identf")
            make_identity(nc, identf[:])

            # decoder staging tiles: L1=[v|0|0], L2=[w|v|0], L3=[0|w|cst]
            # (zero/const columns written once; v/w columns per gen step)
            xg = cpool.tile([20, N], bf16, tag="xg", name="xg")
            xgf = cpool.tile([20, N], f32, tag="xgf", name="xgf")
            L1, L2, L3 = [], [], []
            for j in range(NT):
                for name, lst in (("L1", L1), ("L2", L2), ("L3", L3)):
                    lt = cpool.tile([128, 20], bf16, tag=f"{name}_{j}",
                                    name=f"{name}_{j}")
                    nc.gpsimd.memset(lt[:], 0.0)
                    lst.append(lt)
                nc.vector.tensor_copy(
                    L3[j][:, 16:20], pk128[:, 4 * N + 4 * j:4 * N + 4 * j + 4])

            consts = dict(pk20=pk20, whhT=whhT, wdecT=wdecT, pk128=pk128,
                          pkf32=pkf32,
                          bias=bias, ident=ident, identf=identf, xg=xg,
                          xgf=xgf, out_d=out_d, L1=L1, L2=L2, L3=L3)

            h_tiles, c_prev = None, None
            for t in range(NG):
                if t == K:
                    nc.sync.dma_start(hdbg_d, h_tiles[:])
                c_prev, h_tiles = _emit_step(nc, pools, consts, t, h_tiles,
                                             c_prev, dpool)
    nc.compile()
    return nc


def _host_tensors(inputs):
    """All host-side preprocessing: A matrix, weight composition, per-core shards."""
    f32 = np.float32
    kg = np.asarray(inputs["known_graphs"], f32)
    ei = np.asarray(inputs["edge_index"])
    W_enc_l = np.asarray(inputs["W_enc_l"], f32)
    b_enc_l = np.asarray(inputs["b_enc_l"], f32)
    W_enc_r = np.asarray(inputs["W_enc_r"], f32)
    pos = np.asarray(inputs["pos_emb"], f32)
    W_ih = np.asarray(inputs["W_ih"], f32)
    W_hh = np.asarray(inputs["W_hh"], f32)
    b_ih = np.asarray(inputs["b_ih"], f32)
    b_hh = np.asarray(inputs["b_hh"], f32)
    W_dec_l = np.asarray(inputs["W_dec_l"], f32)
    b_dec_l = np.asarray(inputs["b_dec_l"], f32)
    W_dec_r = np.asarray(inputs["W_dec_r"], f32)

    src, dst = np.asarray(ei[0]), np.asarray(ei[1])
    C = np.zeros((N, N), np.float64)
    np.add.at(C, (dst, src), 1.0)
    cnt = C.sum(1)
    A = (C / np.maximum(cnt, 1.0)[:, None]).astype(f32)

    c64 = np.float64
    Wc1 = W_ih.astype(c64) @ W_enc_l.astype(c64)          # [4H, NF]
    Wc2 = W_ih.astype(c64) @ W_enc_r.astype(c64)
    Wc = np.concatenate([Wc1, Wc2], 1)                    # [4H, 20]
    # bias_t = W_ih @ (b_enc_l + pe_t) + b_ih + b_hh  -> [NG, 4H]
    bias_all = (W_ih.astype(c64) @ (b_enc_l.astype(c64)[:, None] + pos.astype(c64).T)).T \
        + b_ih.astype(c64) + b_hh.astype(c64)
    bias_all = bias_all.astype(f32)
    # decoder pe folds: [16, GEN].  x4096: the whole pre-activation domain is
    # scaled by 64 (h) x 64 (fp8 weights) to stay in fp8e4m3's normal range.
    SC = 64.0
    qr = (np.concatenate([
        (pos[K:NG].astype(c64) @ W_dec_l.T.astype(c64)).T,
        (pos[K:NG].astype(c64) @ W_dec_r.T.astype(c64)).T
        + b_dec_l.astype(c64)[:, None],
    ], 0) * SC).astype(f32)

    # warm-up rhs20: [20, K*N], col index t*N + i
    mean_w = np.einsum("ij,tjf->tif", A.astype(c64), kg.astype(c64))  # [K, N, NF]
    r20w = np.concatenate([
        np.transpose(mean_w, (2, 0, 1)).reshape(NF, -1),
        np.transpose(kg.astype(c64), (2, 0, 1)).reshape(NF, -1),
    ], 0).astype(f32)

    # DoubleRow pair packing: [KT/2 * 128, 2*cols], row j*128+p holds
    # global k-tiles (2j, 2j+1) side by side along the free dim
    def pack_pairs(wT):  # wT [H, cols] -> [H/2, 2*cols]
        cols = wT.shape[1]
        return np.ascontiguousarray(
            wT.reshape(KT // 2, 2, 128, cols).transpose(0, 2, 1, 3)
            .reshape(H // 2, 2 * cols))

    def to_sbuf_cols(x, rows=128):
        # [n*rows, cols] -> [rows, n*cols]: row-block k becomes column-block k
        n = x.shape[0] // rows
        return np.ascontiguousarray(
            x.reshape(n, rows, x.shape[1]).transpose(1, 0, 2)
            .reshape(rows, n * x.shape[1]))

    st2 = kg[-1, :, :2]                                    # [N, 2]
    a_st2 = (A.astype(c64) @ st2.astype(c64)).astype(f32)  # [N, 2]
    A2 = (A.astype(c64) @ A.astype(c64)).astype(f32)

    # xg row order: [mean f2:10 | root f2:10 | mean f0:2 | root f0:2];
    # the same permutation applies to the wc rows and (for the warmup GEMM,
    # where the sum over rows is order-invariant) to r20w.
    perm = list(range(2, 10)) + list(range(12, 20)) + [0, 1, 10, 11]

    # pk128: at(2N) | a2t(2N) | cst(8)  [128, 4N+8] bf16
    cst = np.zeros((128, 8), f32)
    for j in range(NT):
        cst[:, 4 * j:4 * j + 2] = SC * a_st2[128 * j:128 * (j + 1)]
        cst[:, 4 * j + 2:4 * j + 4] = SC * st2[128 * j:128 * (j + 1)]
    pk128 = np.concatenate(
        [to_sbuf_cols(np.ascontiguousarray(A.T)),
         to_sbuf_cols(np.ascontiguousarray(A2.T)), cst], 1).astype(BF)

    # pkf32: st2 cols(4) | qr(GEN cols, rows 0:16)
    pkf32 = np.zeros((128, 4 + GEN), f32)
    pkf32[:, 0:2] = st2[:128]
    pkf32[:, 2:4] = st2[128:]
    pkf32[0:16, 4:4 + GEN] = qr

    wdecT = np.concatenate([W_dec_l, W_dec_r], 0).T * 64.0  # [H, 16]
    wdec_packed = to_sbuf_cols(pack_pairs(wdecT)).astype(F8)  # [128, 8*32]

    shared = {
        "pk128": pk128,
        "pkf32": pkf32,
    }

    in_maps = []
    for c in range(NCORES):
        idx = np.concatenate([np.arange(g * H + c * HS, g * H + (c + 1) * HS)
                              for g in range(4)])
        whhT = pack_pairs(W_hh[idx, :].T * 64.0).astype(F8)           # [H/2, 2GD]
        pkf8 = np.concatenate([to_sbuf_cols(whhT), wdec_packed], 1)
        wcT = np.ascontiguousarray(Wc[idx, :].T[perm]).astype(BF)     # [20, GD]
        pk20 = np.concatenate([wcT, (SC * r20w[perm]).astype(BF)], 1)
        bc = bias_all[:, idx].T                                       # [GD, NG]
        bt = np.ascontiguousarray(
            bc.reshape(MT, 128, NG).transpose(1, 0, 2).reshape(128, MT * NG))
        in_maps.append({
            "pkf8": pkf8, "pk20": pk20, "biases": bt, **shared,
        })
    return in_maps


def kernel(**inputs):
    if _PROG[0] is None:
        _PROG[0] = _build_program()
    nc = _PROG[0]
    in_maps = _host_tensors(inputs)
    res = bass_utils.run_bass_kernel_spmd(
        nc, in_maps, core_ids=list(range(NCORES)))
    kernel.hdbg = np.ascontiguousarray(res.results[0]["hdbg"])
    return np.ascontiguousarray(res.results[0]["gen"]).astype(np.float32)


# exposed for test.py profiling
def run_profiled(inputs, **kwargs):
    if _PROG[0] is None:
        _PROG[0] = _build_program()
    in_maps = _host_tensors(inputs)
    return bass_utils.run_bass_kernel_spmd(
        _PROG[0], in_maps, core_ids=list(range(NCORES)), **kwargs)


# revision 50
# speedup vs baseline: 1.0119x; 1.0048x over previous
"""Trainium2 Bass kernel: autoregressive graph generator (GNN encoder + LSTM + GNN decoder).

Sharding: 8-way tensor parallel over the LSTM hidden/gate dim. Each core holds
1/8 of the gate rows of W_hh (fp8, SBUF-resident) and computes its slice of the
gates; h is AllGathered (fp8) every step. The encoder SAGEConv (NF=10 -> H=2048)
composed with W_ih factors through a rank-20 bottleneck, so W_ih @ W_enc_{l,r} is
precomposed on the host and the whole x-side becomes a K=20 GEMM per step.
The mean aggregation is a fixed dense matrix A built from edge_index on the host.

Step-level schedule (the collective serializes the recurrence, so the focus is
the ~12us comm window): the gen-step decoder is algebraically folded so
m10 = v'(A^2)^T + w'A^T and x10 = v'A^T + w' come straight from the transposed
decoder output (no x_next round trip), and its x-side contribution is a single
20-row GEMM emitted AFTER the W_hh GEMMs so the whole decoder hides behind the
recurrent matmuls. A scalar-paced dummy-matmul keepalive chain runs through each
collective window to keep the PE HAM clock-gate warm. Constant loads are batched
into 5 DMAs, and the gathered-h HBM->SBUF copies go on two engine queues.

All layouts on device are "T-layout": [feature/hidden dim (partitions), nodes (free)].
"""

import numpy as np
import ml_dtypes

import concourse.mybir as mybir
import concourse.tile as tile
from concourse import bacc, bass_utils
from concourse.bass import ts
from concourse.masks import make_identity

BF = ml_dtypes.bfloat16
F8 = ml_dtypes.float8_e4m3

N, NF, H, NG, K = 256, 10, 2048, 20, 10
NCORES = 8
HS = H // NCORES          # 256 hidden dims per core
GD = 4 * HS               # 1024 gate rows per core
MT = GD // 128            # 8 gate m-tiles per core
KT = H // 128             # 16 h k-tiles
NT = N // 128             # 2 node tiles
GEN = NG - K              # 10 generated steps
WOFF = 8 * 2 * GD         # wdec offset inside pkf8

_PROG = [None]


def _emit_step(nc, pools, consts, t, h_tiles, c_prev, dpool):
    """One LSTM step: gate GEMMs + cell update + AllGather trigger + keepalive.

    hbig k-tile pair j = core j's h2 slice (DoubleRow pairs (2j, 2j+1)).
    Gen steps: decoder front (v' GEMM + transposes) first, then the 64 W_hh
    DR matmuls with the decoder tail (m10/x10) interleaved mid-burst, then a
    single 20-row x-side GEMM, so decoder latency hides behind the W_hh GEMMs.
    Returns (c_new, hbig_for_next_step).
    """
    f32, bf16 = mybir.dt.float32, mybir.dt.bfloat16
    fp8 = mybir.dt.float8e4
    cpool, wpool, apool, gpool, spool = pools
    pk20, pk128, pkf32, bias = (consts["pk20"], consts["pk128"],
                                consts["pkf32"], consts["bias"])
    whhT, wdecT = consts["whhT"], consts["wdecT"]
    ident, xg, out_d = consts["ident"], consts["xg"], consts["out_d"]
    Sig = mybir.ActivationFunctionType.Sigmoid
    Tanh = mybir.ActivationFunctionType.Tanh
    have_h = t > 0
    m_order = [0, 2, 4, 6, 1, 3, 5, 7]  # finish hidden-half 0 (i,f,g,o) early
    DR = mybir.MatmulPerfMode.DoubleRow

    def wc_m(m):
        return pk20[0:20, m * 128:(m + 1) * 128]

    def whh_j(j, m):
        return whhT[j][:].rearrange("p (s w) -> p s w", s=2)[:, :, ts(m, 128)]

    def wdec_j(j):
        return wdecT[j][:].rearrange("p (s w) -> p s w", s=2)

    def at_k(k):
        return pk128[:, ts(k, N)]

    def a2t_k(k):
        return pk128[:, ts(2 + k, N)]

    def hpair(j):
        q, r = divmod(j, 2)
        return h_tiles[q][:].rearrange("p (a n) -> p a n", a=KT // 4)[
            :, 2 * r:2 * r + 2, :]
    gen = t >= K
    vw_sb, vwT = None, None

    if gen:
        s = t - K
        L1, L2, L3 = consts["L1"], consts["L2"], consts["L3"]
        # decoder front: v'|w' = Wdec^T h  [16, N], then +qr, then transpose
        vw_ps = spool.tile([16, N], f32, tag="sp", name=f"vwps{t}")
        for j in range(KT // 2):
            nc.tensor.matmul(vw_ps[:], wdec_j(j), hpair(j),
                             start=(j == 0), stop=(j == KT // 2 - 1),
                             perf_mode=DR)
        vw_sb = wpool.tile([16, N], bf16, tag="vw", name=f"vwsb{t}")
        nc.vector.tensor_scalar_add(vw_sb[:], vw_ps[:],
                                    pkf32[0:16, 4 + s:5 + s])
        # scatter v'/w' transposes into the L staging tiles (column slices
        # only -- partition-aligned).  L1=[v|0|0], L2=[w|v|0], L3=[0|w|cst].
        for j in range(NT):
            tp = spool.tile([128, 16], bf16, tag="sp", name=f"vwT{t}_{j}")
            nc.tensor.transpose(tp[:], vw_sb[:, ts(j, 128)], ident[:16, :16])
            nc.vector.tensor_copy(L1[j][:, 0:8], tp[:, 0:8])
            nc.vector.tensor_copy(L2[j][:, 0:8], tp[:, 8:16])
            nc.vector.tensor_copy(L2[j][:, 8:16], tp[:, 0:8])
            nc.vector.tensor_copy(L3[j][:, 8:16], tp[:, 8:16])

    # recurrent gate GEMMs.  PSUM banks pair m-tiles that finish
    # consecutively in m_order ((0,2),(4,6),(1,3),(5,7)) so the hh=0
    # activations fire as soon as their bank closes.
    # Warmup: all 8 x-side MMs (host-precomputed r20w, no h dependency) are
    # emitted BEFORE the W_hh loop so they execute inside the collective
    # window while the PE is otherwise idle.  Gen: W_hh first (start),
    # x-side last (stop) once the decoder tail lands.
    gp = {}
    for mi, m in enumerate(m_order):
        g_t = gpool.tile([128, N], f32, tag="gp", name=f"gp{t}_{m}")
        gp[m] = g_t[:]
        g = gp[m]
        if not gen:
            nc.tensor.matmul(g, wc_m(m),
                             pk20[0:20, GD + t * N:GD + (t + 1) * N],
                             start=True, stop=not have_h)
        if have_h:
            for j in range(KT // 2):
                nc.tensor.matmul(
                    g, whh_j(j, m), hpair(j),
                    start=gen and (j == 0),
                    stop=(not gen) and (j == KT // 2 - 1),
                    perf_mode=DR)
        if gen and mi == 3:
            # decoder tail mid-burst, one [20, N] accumulation:
            # rows 0:8  = m10' = v'(A^2)^T + w'A^T      (L1 x a2t + L2 x at)
            # rows 8:16 = x10' = v'A^T + w'             (L2 x at + L3 x ident)
            # rows 16:20 = [(A@st2)^T ; st2^T]          (L3 const cols x ident)
            L1, L2, L3 = consts["L1"], consts["L2"], consts["L3"]
            out20 = spool.tile([20, N], f32, tag="sp", name=f"o20{t}")
            for j in range(NT):
                nc.tensor.matmul(out20[:], L1[j][:], a2t_k(j),
                                 start=(j == 0), stop=False)
                nc.tensor.matmul(out20[:], L2[j][:], at_k(j),
                                 start=False, stop=False)
            for j in range(NT):
                nc.tensor.matmul(out20[:, ts(j, 128)], L3[j][:], ident[:],
                                 start=False, stop=(j == NT - 1))
            nc.vector.tensor_copy(xg[:], out20[:])
            # f32 copy for the output path (avoids bf16-rounding the result)
            xgf = consts["xgf"]
            nc.vector.tensor_copy(xgf[:], out20[:])

    if gen:
        for m in m_order:
            nc.tensor.matmul(gp[m], wc_m(m), xg[:],
                             start=False, stop=True)

    h2 = apool.tile([128, 2 * N], fp8, tag="h2", name=f"h2_{t}")
    hb = None
    if t < NG - 1:
        # four separate tiles so each quarter-copy's consumers depend only on
        # their own chunk (single-tile hb made every MM wait for all 4 copies)
        hb = [wpool.tile([128, KT * N // 4], fp8, tag=f"hbig{q}",
                         name=f"hbig{t}_{q}") for q in range(4)]
    c_new = []
    for hh in range(2):
        def bcol(m):
            return bias[:, (m * NG + t):(m * NG + t + 1)]
        isc = 1.0 / 64.0     # undo the x64 fp8-normal-range weight scaling
        si = apool.tile([128, N], f32, tag="si", name=f"si{t}_{hh}")
        nc.scalar.activation(si[:], gp[0 + hh], Sig, bias=bcol(0 + hh), scale=isc)
        sf = apool.tile([128, N], f32, tag="sf", name=f"sf{t}_{hh}")
        nc.scalar.activation(sf[:], gp[2 + hh], Sig, bias=bcol(2 + hh), scale=isc)
        tg = apool.tile([128, N], f32, tag="tg", name=f"tg{t}_{hh}")
        nc.scalar.activation(tg[:], gp[4 + hh], Tanh, bias=bcol(4 + hh), scale=isc)
        so = apool.tile([128, N], f32, tag="so", name=f"so{t}_{hh}")
        nc.scalar.activation(so[:], gp[6 + hh], Sig, bias=bcol(6 + hh), scale=isc)

        cn = wpool.tile([128, N], f32, tag=f"c{hh}", name=f"c{t}_{hh}")
        if t == 0:
            nc.vector.tensor_mul(cn[:], si[:], tg[:])          # c = sig(i)*tanh(g)
        else:
            p = apool.tile([128, N], f32, tag="p", name=f"p{t}_{hh}")
            nc.vector.tensor_mul(p[:], si[:], tg[:])
            tmp = apool.tile([128, N], f32, tag="tmp", name=f"tmp{t}_{hh}")
            nc.vector.tensor_mul(tmp[:], sf[:], c_prev[hh][:])
            nc.vector.tensor_add(cn[:], tmp[:], p[:])
        tc2 = apool.tile([128, N], f32, tag="tc", name=f"tc{t}_{hh}")
        nc.scalar.activation(tc2[:], cn[:], Tanh)
        nc.vector.tensor_mul(h2[:, ts(hh, N)], so[:], tc2[:])
        c_new.append(cn)

    if t < NG - 1:
        inb = dpool.tile([N, N], fp8, tag="inb", name=f"inb{t}")
        outb = dpool.tile([H, N], fp8, tag="outb", name=f"outb{t}")
        # per-half input DMAs: the hh=0 half ships while the hh=1
        # elementwise is still running
        for hh in range(2):
            nc.sync.dma_start(inb[ts(hh, 128), :], h2[:, ts(hh, N)])
        nc.gpsimd.collective_compute(
            "AllGather",
            mybir.AluOpType.bypass,
            replica_groups=[list(range(NCORES))],
            ins=[inb.opt()],
            outs=[outb.opt()],
        )
        outb3 = outb.rearrange("(a p) n -> p a n", p=128)
        # gathered-h copies in 4 chunks on two engine queues: the first
        # chunk (pairs j=0,1) lands earlier so the next step's W_hh matmuls
        # start sooner; later chunks stream in behind the consuming j-loop
        engines = [nc.sync, nc.scalar, nc.sync, nc.scalar]
        for q in range(4):
            engines[q].dma_start(
                hb[q][:].rearrange("p (a n) -> p a n", a=KT // 4),
                outb3[:, ts(q, KT // 4), :])

    if gen:
        # output x_next = [st2 | x10'^T], off the critical path
        xgf, identf = consts["xgf"], consts["identf"]
        for j in range(NT):
            xt = spool.tile([128, 20], f32, tag="sp", name=f"xt{t}_{j}")
            nc.tensor.transpose(xt[:], xgf[0:20, ts(j, 128)], identf[:20, :20])
            xn = wpool.tile([128, NF], f32, tag=f"xn{j}", name=f"xn{t}_{j}")
            nc.vector.tensor_copy(xn[:, 0:2], pkf32[:, 2 * j:2 * j + 2])
            nc.scalar.activation(xn[:, 2:NF], xt[:, 8:16],
                                 mybir.ActivationFunctionType.Copy,
                                 scale=1.0 / 64.0)
            nc.sync.dma_start(out_d[t - K, ts(j, 128), :], xn[:])
    return c_new, hb


def _build_program():
    f32, bf16 = mybir.dt.float32, mybir.dt.bfloat16
    fp8 = mybir.dt.float8e4
    nc = bacc.Bacc("TRN2", target_bir_lowering=False, debug=False,
                   num_devices=NCORES)

    # batched constant inputs (few big DMAs instead of ~26 serialized triggers)
    pk20_d = nc.dram_tensor("pk20", [20, GD + K * N], bf16,
                            kind="ExternalInput").ap()          # wc | r20w
    bias_d = nc.dram_tensor("biases", [128, MT * NG], f32,
                            kind="ExternalInput").ap()
    pkf8_d = nc.dram_tensor("pkf8", [128, WOFF + 8 * 32], fp8,
                            kind="ExternalInput").ap()          # whh | wdec
    pk128_d = nc.dram_tensor("pk128", [128, 4 * N + 8], bf16,
                             kind="ExternalInput").ap()         # at | a2t | cst
    pkf32_d = nc.dram_tensor("pkf32", [128, 4 + GEN], f32,
                             kind="ExternalInput").ap()         # st2 | qr
    out_d = nc.dram_tensor("gen", [GEN, N, NF], f32, kind="ExternalOutput").ap()
    hdbg_d = nc.dram_tensor("hdbg", [128, KT * N // 4], fp8,
                            kind="ExternalOutput").ap()

    with tile.TileContext(nc) as tc:
        with (
            tc.tile_pool(name="const", bufs=1) as cpool,
            tc.tile_pool(name="work", bufs=2) as wpool,
            tc.tile_pool(name="act", bufs=3) as apool,
            tc.tile_pool(name="gp", bufs=6, space="PSUM") as gpool,
            tc.tile_pool(name="sp", bufs=2, space="PSUM") as spool,
            tc.tile_pool(name="dram", bufs=2, space="DRAM") as dpool,
        ):
            pools = (cpool, wpool, apool, gpool, spool)

            # ---- constants, batched.  Order: step-0 needs pk20+bias first.
            pk20 = cpool.tile([20, GD + K * N], bf16, tag="pk20", name="pk20")
            nc.sync.dma_start(pk20[:], pk20_d[:])
            bias = cpool.tile([128, MT * NG], f32, tag="bias", name="bias")
            nc.sync.dma_start(bias[:], bias_d[:])
            whhT = []
            for k in range(KT // 2):
                w = cpool.tile([128, 2 * GD], fp8, tag=f"whh{k}", name=f"whh{k}")
                nc.scalar.dma_start(w[:], pkf8_d[:, k * 2 * GD:(k + 1) * 2 * GD])
                whhT.append(w)
            wdecT = []
            for k in range(KT // 2):
                w = cpool.tile([128, 32], fp8, tag=f"wdec{k}", name=f"wdec{k}")
                nc.scalar.dma_start(
                    w[:], pkf8_d[:, WOFF + 32 * k:WOFF + 32 * (k + 1)])
                wdecT.append(w)
            pk128 = cpool.tile([128, 4 * N + 8], bf16, tag="pk128",
                               name="pk128")
            nc.sync.dma_start(pk128[:], pk128_d[:])
            pkf32 = cpool.tile([128, 4 + GEN], f32, tag="pkf32",
                               name="pkf32")
            nc.sync.dma_start(pkf32[:], pkf32_d[:])

            ident = cpool.tile([128, 128], bf16, tag="ident", name="ident")
            make_identity(nc, ident[:])
            identf = cpool.tile([32, 32], f32, tag="identf", name="identf")
            make_identity(nc, identf[:])

            # decoder staging tiles: L1=[v|0|0], L2=[w|v|0], L3=[0|w|cst]
            # (zero/const columns written once; v/w columns per gen step)
            xg = cpool.tile([20, N], bf16, tag="xg", name="xg")
            xgf = cpool.tile([20, N], f32, tag="xgf", name="xgf")
            L1, L2, L3 = [], [], []
            for j in range(NT):
                for name, lst in (("L1", L1), ("L2", L2), ("L3", L3)):
                    lt = cpool.tile([128, 20], bf16, tag=f"{name}_{j}",
                                    name=f"{name}_{j}")
                    nc.gpsimd.memset(lt[:], 0.0)
                    lst.append(lt)
                nc.vector.tensor_copy(
                    L3[j][:, 16:20], pk128[:, 4 * N + 4 * j:4 * N + 4 * j + 4])

            consts = dict(pk20=pk20, whhT=whhT, wdecT=wdecT, pk128=pk128,
                          pkf32=pkf32,
                          bias=bias, ident=ident, identf=identf, xg=xg,
                          xgf=xgf, out_d=out_d, L1=L1, L2=L2, L3=L3)

            h_tiles, c_prev = None, None
            for t in range(NG):
                if t == K:
                    nc.sync.dma_start(hdbg_d, h_tiles[0][:])
                c_prev, h_tiles = _emit_step(nc, pools, consts, t, h_tiles,
                                             c_prev, dpool)
    nc.compile()
    return nc


def _host_tensors(inputs):
    """All host-side preprocessing: A matrix, weight composition, per-core shards."""
    f32 = np.float32
    kg = np.asarray(inputs["known_graphs"], f32)
    ei = np.asarray(inputs["edge_index"])
    W_enc_l = np.asarray(inputs["W_enc_l"], f32)
    b_enc_l = np.asarray(inputs["b_enc_l"], f32)
    W_enc_r = np.asarray(inputs["W_enc_r"], f32)
    pos = np.asarray(inputs["pos_emb"], f32)
    W_ih = np.asarray(inputs["W_ih"], f32)
    W_hh = np.asarray(inputs["W_hh"], f32)
    b_ih = np.asarray(inputs["b_ih"], f32)
    b_hh = np.asarray(inputs["b_hh"], f32)
    W_dec_l = np.asarray(inputs["W_dec_l"], f32)
    b_dec_l = np.asarray(inputs["b_dec_l"], f32)
    W_dec_r = np.asarray(inputs["W_dec_r"], f32)

    src, dst = np.asarray(ei[0]), np.asarray(ei[1])
    C = np.zeros((N, N), np.float64)
    np.add.at(C, (dst, src), 1.0)
    cnt = C.sum(1)
    A = (C / np.maximum(cnt, 1.0)[:, None]).astype(f32)

    c64 = np.float64
    Wc1 = W_ih.astype(c64) @ W_enc_l.astype(c64)          # [4H, NF]
    Wc2 = W_ih.astype(c64) @ W_enc_r.astype(c64)
    Wc = np.concatenate([Wc1, Wc2], 1)                    # [4H, 20]
    # bias_t = W_ih @ (b_enc_l + pe_t) + b_ih + b_hh  -> [NG, 4H]
    bias_all = (W_ih.astype(c64) @ (b_enc_l.astype(c64)[:, None] + pos.astype(c64).T)).T \
        + b_ih.astype(c64) + b_hh.astype(c64)
    bias_all = bias_all.astype(f32)
    # decoder pe folds: [16, GEN].  x4096: the whole pre-activation domain is
    # scaled by 64 (h) x 64 (fp8 weights) to stay in fp8e4m3's normal range.
    SC = 64.0
    qr = (np.concatenate([
        (pos[K:NG].astype(c64) @ W_dec_l.T.astype(c64)).T,
        (pos[K:NG].astype(c64) @ W_dec_r.T.astype(c64)).T
        + b_dec_l.astype(c64)[:, None],
    ], 0) * SC).astype(f32)

    # warm-up rhs20: [20, K*N], col index t*N + i
    mean_w = np.einsum("ij,tjf->tif", A.astype(c64), kg.astype(c64))  # [K, N, NF]
    r20w = np.concatenate([
        np.transpose(mean_w, (2, 0, 1)).reshape(NF, -1),
        np.transpose(kg.astype(c64), (2, 0, 1)).reshape(NF, -1),
    ], 0).astype(f32)

    # DoubleRow pair packing: [KT/2 * 128, 2*cols], row j*128+p holds
    # global k-tiles (2j, 2j+1) side by side along the free dim
    def pack_pairs(wT):  # wT [H, cols] -> [H/2, 2*cols]
        cols = wT.shape[1]
        return np.ascontiguousarray(
            wT.reshape(KT // 2, 2, 128, cols).transpose(0, 2, 1, 3)
            .reshape(H // 2, 2 * cols))

    def to_sbuf_cols(x, rows=128):
        # [n*rows, cols] -> [rows, n*cols]: row-block k becomes column-block k
        n = x.shape[0] // rows
        return np.ascontiguousarray(
            x.reshape(n, rows, x.shape[1]).transpose(1, 0, 2)
            .reshape(rows, n * x.shape[1]))

    st2 = kg[-1, :, :2]                                    # [N, 2]
    a_st2 = (A.astype(c64) @ st2.astype(c64)).astype(f32)  # [N, 2]
    A2 = (A.astype(c64) @ A.astype(c64)).astype(f32)

    # xg row order: [mean f2:10 | root f2:10 | mean f0:2 | root f0:2];
    # the same permutation applies to the wc rows and (for the warmup GEMM,
    # where the sum over rows is order-invariant) to r20w.
    perm = list(range(2, 10)) + list(range(12, 20)) + [0, 1, 10, 11]

    # pk128: at(2N) | a2t(2N) | cst(8)  [128, 4N+8] bf16
    cst = np.zeros((128, 8), f32)
    for j in range(NT):
        cst[:, 4 * j:4 * j + 2] = SC * a_st2[128 * j:128 * (j + 1)]
        cst[:, 4 * j + 2:4 * j + 4] = SC * st2[128 * j:128 * (j + 1)]
    pk128 = np.concatenate(
        [to_sbuf_cols(np.ascontiguousarray(A.T)),
         to_sbuf_cols(np.ascontiguousarray(A2.T)), cst], 1).astype(BF)

    # pkf32: st2 cols(4) | qr(GEN cols, rows 0:16)
    pkf32 = np.zeros((128, 4 + GEN), f32)
    pkf32[:, 0:2] = st2[:128]
    pkf32[:, 2:4] = st2[128:]
    pkf32[0:16, 4:4 + GEN] = qr

    wdecT = np.concatenate([W_dec_l, W_dec_r], 0).T * 64.0  # [H, 16]
    wdec_packed = to_sbuf_cols(pack_pairs(wdecT)).astype(F8)  # [128, 8*32]

    shared = {
        "pk128": pk128,
        "pkf32": pkf32,
    }

    in_maps = []
    for c in range(NCORES):
        idx = np.concatenate([np.arange(g * H + c * HS, g * H + (c + 1) * HS)
                              for g in range(4)])
        whhT = pack_pairs(W_hh[idx, :].T * 64.0).astype(F8)           # [H/2, 2GD]
        pkf8 = np.concatenate([to_sbuf_cols(whhT), wdec_packed], 1)
        wcT = np.ascontiguousarray(Wc[idx, :].T[perm]).astype(BF)     # [20, GD]
        pk20 = np.concatenate([wcT, (SC * r20w[perm]).astype(BF)], 1)
        bc = bias_all[:, idx].T                                       # [GD, NG]
        bt = np.ascontiguousarray(
            bc.reshape(MT, 128, NG).transpose(1, 0, 2).reshape(128, MT * NG))
        in_maps.append({
            "pkf8": pkf8, "pk20": pk20, "biases": bt, **shared,
        })
    return in_maps


def kernel(**inputs):
    if _PROG[0] is None:
        _PROG[0] = _build_program()
    nc = _PROG[0]
    in_maps = _host_tensors(inputs)
    res = bass_utils.run_bass_kernel_spmd(
        nc, in_maps, core_ids=list(range(NCORES)))
    kernel.hdbg = np.ascontiguousarray(res.results[0]["hdbg"])
    return np.ascontiguousarray(res.results[0]["gen"]).astype(np.float32)


# exposed for test.py profiling
def run_profiled(inputs, **kwargs):
    if _PROG[0] is None:
        _PROG[0] = _build_program()
    in_maps = _host_tensors(inputs)
    return bass_utils.run_bass_kernel_spmd(
        _PROG[0], in_maps, core_ids=list(range(NCORES)), **kwargs)
